# revision 1
# baseline (speedup 1.0000x reference)
"""GQA attention kernel for Trainium2, 8-core tensor-parallel.

Sharding: 8 cores = 2 batches x 4 KV-groups. Each core handles one
(batch, kv_group): projections for its 4 Q-heads + 1 KV-head, RoPE,
causal attention, and its row-shard of Wo -> partial [T, D] output.
Host sums the 4 partials per batch (the Wo all-reduce) at unshard.

Attention runs in transposed orientation: Q^T/K^T live as [HD, T] so
S^T tiles [s,q] come from single K=128 matmuls; softmax row-sums are
computed by an all-ones [128,128] stationary matmul per s-chunk (which
also broadcasts the sums across partitions); 1/sigma (fast approx
reciprocal) scales each head's O^T before the output projection.
Matmuls run in float32r (~2e-4 rel err at ~233ns per 128x128x512 MM).
"""
from contextlib import ExitStack

import numpy as np

import concourse.bass as bass
import concourse.mybir as mybir
import concourse.tile as tile
from concourse import bacc
from concourse.bass_utils import run_bass_kernel_spmd

B, T, D = 2, 2048, 2048
H, KV, HD = 16, 4, 128
R = H // KV                  # 4 query heads per kv head (per core)
GC = R * HD                  # 512 query-proj cols per core
THETA = 10000.0
TQ = 512                     # q-tile size
NJ = T // TQ                 # 4 q-tiles
ND = D // 128                # 16 contraction chunks
SCALE = float(HD) ** -0.5

F32 = mybir.dt.float32
MM_DT = mybir.dt.float32r
MM_NP = np.float32
BF16 = mybir.dt.bfloat16
AF = mybir.ActivationFunctionType

_CACHED_NC = None


def _build_nc():
    nc = bacc.Bacc("TRN2", target_bir_lowering=False, debug=False, num_devices=8)

    xT = nc.dram_tensor("xT", [D, T], MM_DT, kind="ExternalInput").ap()
    wq = nc.dram_tensor("wq", [128, ND * GC], MM_DT, kind="ExternalInput").ap()
    wk = nc.dram_tensor("wk", [128, ND * HD], MM_DT, kind="ExternalInput").ap()
    wv = nc.dram_tensor("wv", [128, ND * HD], MM_DT, kind="ExternalInput").ap()
    wo = nc.dram_tensor("wo", [128, R * D], MM_DT, kind="ExternalInput").ap()
    cosT = nc.dram_tensor("cosT", [HD, T], F32, kind="ExternalInput").ap()
    sinT = nc.dram_tensor("sinT", [HD, T], F32, kind="ExternalInput").ap()
    masks = nc.dram_tensor("masks", [128, 4 * TQ], BF16, kind="ExternalInput").ap()
    out = nc.dram_tensor("out", [T, D], F32, kind="ExternalOutput").ap()

    with tile.TileContext(nc) as tc, ExitStack() as ctx:
        res = ctx.enter_context(tc.tile_pool(name="res", bufs=1))
        sb = ctx.enter_context(tc.tile_pool(name="sb", bufs=2))
        pp = ctx.enter_context(tc.tile_pool(name="pp", bufs=2, space="PSUM"))

        # ---- resident weights / tables ----
        # xt/cos/sin stream on the sync queue; weights go on the scalar /
        # vector engines' queues so j=0's activations aren't stuck behind
        # 10MB of weight DMA.
        # single sync queue, strict priority order: transfers stripe across
        # all 16 DMA engines at full HBM BW, so queue order = arrival order.
        xts0 = []
        for d in range(4):
            xt = sb.tile([128, TQ], MM_DT, tag="xt", bufs=16, name=f"xt0_{d}")
            nc.sync.dma_start(xt[:], xT[d * 128:(d + 1) * 128, 0:TQ])
            xts0.append(xt)
        wk_sb = res.tile([128, ND * HD], MM_DT)
        nc.sync.dma_start(wk_sb[:], wk[:])
        for d in range(4, ND):
            xt = sb.tile([128, TQ], MM_DT, tag="xt", bufs=16, name=f"xt0_{d}")
            nc.sync.dma_start(xt[:], xT[d * 128:(d + 1) * 128, 0:TQ])
            xts0.append(xt)
        cosj0 = sb.tile([128, TQ], F32, tag="cos", bufs=1, name="cosj0")
        nc.sync.dma_start(cosj0[:], cosT[:, 0:TQ])
        sinj0 = sb.tile([128, TQ], F32, tag="sin", bufs=1, name="sinj0")
        nc.sync.dma_start(sinj0[:], sinT[:, 0:TQ])
        wv_sb = res.tile([128, ND * HD], MM_DT)
        nc.sync.dma_start(wv_sb[:], wv[:])
        wq_sb = res.tile([128, ND * GC], MM_DT)    # chunk d at cols [d*GC, (d+1)*GC)
        nc.sync.dma_start(wq_sb[:], wq[:])
        mask_sb = res.tile([128, 4 * TQ], BF16)
        nc.sync.dma_start(mask_sb[:], masks[:])
        wo_sb = res.tile([128, R * D], MM_DT)      # head h rows at cols [h*D, (h+1)*D)
        nc.sync.dma_start(wo_sb[:], wo[:])
        kT_sb = res.tile([128, T], MM_DT)          # K^T resident, filled per j
        v_sb = res.tile([128, T], MM_DT)           # V natural, chunk c at cols c*128
        ident = res.tile([128, 128], F32)
        from concourse.masks import make_identity
        make_identity(nc, ident[:])
        ones_f = res.tile([128, 128], F32)
        nc.vector.memset(ones_f[:], 1.0)
        ones_c = res.tile([128, 128], MM_DT)       # sigma-reduce+broadcast stationary
        nc.vector.tensor_copy(ones_c[:], ones_f[:])

        def rope(dst, ps, cosj, sinj):
            # dst = ps * cos + rotate_half(ps) * sin   (partition dim = head dim)
            rot = sb.tile([128, TQ], F32, tag="rot", bufs=2)
            nc.scalar.mul(rot[0:64, :], ps[64:128, :], -1.0)
            nc.scalar.copy(rot[64:128, :], ps[0:64, :])
            tmp = sb.tile([128, TQ], F32, tag="ropetmp", bufs=2)
            nc.vector.tensor_mul(tmp[:], rot[:], sinj[:])
            nc.vector.tensor_mul(dst, ps[:], cosj[:])
            nc.vector.tensor_add(dst, dst.bitcast(F32), tmp[:])

        for j in range(NJ):
            q0 = j * TQ
            # ---- stage inputs for this q/s tile ----
            if j == 0:
                xts, cosj, sinj = xts0, cosj0, sinj0
            else:
                xts = []
                for d in range(ND):
                    xt = sb.tile([128, TQ], MM_DT, tag="xt", bufs=16)
                    nc.sync.dma_start(xt[:], xT[d * 128:(d + 1) * 128, q0:q0 + TQ])
                    xts.append(xt)
                cosj = sb.tile([128, TQ], F32, tag="cos", bufs=1)
                nc.sync.dma_start(cosj[:], cosT[:, q0:q0 + TQ])
                sinj = sb.tile([128, TQ], F32, tag="sin", bufs=1)
                nc.sync.dma_start(sinj[:], sinT[:, q0:q0 + TQ])

            # ---- A1: K^T and V^T for s-tile j ----
            k_ps = pp.tile([128, TQ], F32, tag="pa", bufs=2)
            vt_ps = pp.tile([128, TQ], F32, tag="pa", bufs=2)
            for d in range(ND):
                nc.tensor.matmul(k_ps[:], wk_sb[:, d * HD:(d + 1) * HD], xts[d][:],
                                 start=(d == 0), stop=(d == ND - 1))
            for d in range(ND):
                nc.tensor.matmul(vt_ps[:], wv_sb[:, d * HD:(d + 1) * HD], xts[d][:],
                                 start=(d == 0), stop=(d == ND - 1))
            rope(kT_sb[:, q0:q0 + TQ], k_ps, cosj, sinj)
            vt_sbt = sb.tile([128, TQ], F32, tag="vtsb", bufs=2)
            nc.vector.tensor_copy(vt_sbt[:], vt_ps[:])
            for c4 in range(4):
                ptt = pp.tile([128, 128], F32, tag="pc", bufs=2)
                nc.tensor.transpose(ptt[:], vt_sbt[:, c4 * 128:(c4 + 1) * 128], ident[:])
                nc.vector.tensor_copy(v_sb[:, (4 * j + c4) * 128:(4 * j + c4 + 1) * 128], ptt[:])

            # ---- A2: Q^T per head + rope ----
            q_tiles = []
            for h in range(R):
                q_ps = pp.tile([128, TQ], F32, tag="pa", bufs=2)
                for d in range(ND):
                    nc.tensor.matmul(
                        q_ps[:], wq_sb[:, d * GC + h * 128:d * GC + (h + 1) * 128],
                        xts[d][:], start=(d == 0), stop=(d == ND - 1))
                qh = sb.tile([128, TQ], MM_DT, tag="qsb", bufs=5)
                rope(qh[:], q_ps, cosj, sinj)
                q_tiles.append(qh)

            # ---- B: causal attention per head ----
            o_tiles = []
            nch = 4 * (j + 1)
            for h in range(R):
                o_ps = pp.tile([128, TQ], F32, tag="po", bufs=2)
                sg_ps = pp.tile([128, TQ], F32, tag="po", bufs=2)
                for c in range(nch):
                    s_ps = pp.tile([128, TQ], F32, tag="ps", bufs=2)
                    nc.tensor.matmul(s_ps[:], kT_sb[:, c * 128:(c + 1) * 128],
                                     q_tiles[h][:], start=True, stop=True)
                    p = sb.tile([128, TQ], MM_DT, tag="psb", bufs=5)
                    nc.scalar.activation(p[:], s_ps[:], AF.Exp, scale=SCALE)
                    if c >= 4 * j:  # diagonal block: apply causal mask
                        m = c - 4 * j
                        nc.vector.tensor_mul(p[:], p[:].bitcast(F32),
                                             mask_sb[:, m * TQ:(m + 1) * TQ])
                    # sigma: ones@p accumulates row-sums broadcast to all parts
                    nc.tensor.matmul(sg_ps[:], ones_c[:], p[:],
                                     start=(c == 0), stop=(c == nch - 1))
                    nc.tensor.matmul(o_ps[:], v_sb[:, c * 128:(c + 1) * 128], p[:],
                                     start=(c == 0), stop=(c == nch - 1))
                sgs = sb.tile([128, TQ], F32, tag="sgs", bufs=2)
                nc.vector.tensor_copy(sgs[:], sg_ps[:])
                rcb = sb.tile([128, TQ], F32, tag="rcb", bufs=2)
                nc.vector.reciprocal_approx_fast(rcb[:], sgs[:])
                oh = sb.tile([128, TQ], MM_DT, tag="osb", bufs=6)
                nc.vector.tensor_mul(oh[:], o_ps[:], rcb[:])
                o_tiles.append(oh)

            # ---- C: output projection for q-tile j ----
            for qs in range(4):
                for n in range(NJ):
                    pc = pp.tile([128, 512], F32, tag="pc", bufs=2)
                    for h in range(R):
                        nc.tensor.matmul(
                            pc[:], o_tiles[h][:, qs * 128:(qs + 1) * 128],
                            wo_sb[:, h * D + n * 512:h * D + (n + 1) * 512],
                            start=(h == 0), stop=(h == R - 1))
                    ob = sb.tile([128, 512], F32, tag="ob", bufs=3)
                    nc.scalar.copy(ob[:], pc[:])
                    nc.gpsimd.dma_start(
                        out[q0 + qs * 128:q0 + (qs + 1) * 128, n * 512:(n + 1) * 512],
                        ob[:])

    nc.compile()
    return nc


def _get_nc():
    global _CACHED_NC
    if _CACHED_NC is None:
        _CACHED_NC = _build_nc()
    return _CACHED_NC


def _rope_tables_T():
    inv_freq = (1.0 / (THETA ** (np.arange(0, HD, 2, dtype=np.float32) / HD))).astype(np.float32)
    pos = np.arange(T, dtype=np.float32)
    freqs = np.outer(pos, inv_freq).astype(np.float32)      # [T, HD/2]
    emb = np.concatenate([freqs, freqs], axis=-1)           # [T, HD]
    return (np.cos(emb).T.astype(np.float32).copy(),
            np.sin(emb).T.astype(np.float32).copy())        # [HD, T]


def _diag_masks():
    # masks[:, m*TQ + jj] for offset delta = m*128: keep jj >= i + delta
    import ml_dtypes
    i = np.arange(128)[:, None]
    jj = np.arange(TQ)[None, :]
    blocks = [(jj >= i + m * 128).astype(ml_dtypes.bfloat16) for m in range(4)]
    return np.concatenate(blocks, axis=1)                   # [128, 4*TQ]


def kernel(x, Wq, Wk, Wv, Wo, _trace=False):
    x = np.asarray(x, dtype=np.float32)
    Wq = np.asarray(Wq, dtype=MM_NP)
    Wk = np.asarray(Wk, dtype=MM_NP)
    Wv = np.asarray(Wv, dtype=MM_NP)
    Wo = np.asarray(Wo, dtype=MM_NP)

    cosT, sinT = _rope_tables_T()
    masks = _diag_masks()
    in_maps = []
    for core in range(8):
        b, g = core // KV, core % KV
        def chunkT(w):  # [ND*128, C] -> [128, ND*C] with chunk d at cols [d*C,(d+1)*C)
            nd = w.shape[0] // 128
            return np.ascontiguousarray(
                w.reshape(nd, 128, -1).transpose(1, 0, 2).reshape(128, -1))
        in_maps.append({
            "xT": np.ascontiguousarray(x[b].T.astype(MM_NP)),
            "wq": chunkT(Wq[:, g * GC:(g + 1) * GC]),
            "wk": chunkT(Wk[:, g * HD:(g + 1) * HD]),
            "wv": chunkT(Wv[:, g * HD:(g + 1) * HD]),
            "wo": chunkT(Wo[g * GC:(g + 1) * GC, :]),
            "cosT": cosT, "sinT": sinT, "masks": masks,
        })

    nc = _get_nc()
    res = run_bass_kernel_spmd(nc, in_maps, core_ids=list(range(8)), trace=_trace)

    outp = np.zeros((B, T, D), dtype=np.float32)
    for core in range(8):
        b = core // KV
        outp[b] += res.results[core]["out"]
    if _trace:
        kernel._last_exec_time_ns = res.exec_time_ns
        kernel._last_trace = res.instructions_and_trace
    return outp



# revision 7
# speedup vs baseline: 1.1167x; 1.1167x over previous
"""GQA attention kernel for Trainium2, 8-core tensor-parallel.

Sharding: 8 cores = 2 batches x 4 KV-groups. Each core handles one
(batch, kv_group): projections for its 4 Q-heads + 1 KV-head, RoPE,
causal attention, and its row-shard of Wo -> partial [T, D] output.
Host sums the 4 partials per batch (the Wo all-reduce) at unshard.

v2: all matmuls in bf16 (216ns vs 233ns per 128x128x512 MM, FWL weight
loads 97ns vs 187ns). Softmax row-sums (sigma) accumulate on the vector
engine (bf16 2x mode) instead of one ones-matmul per chunk, saving ~144
PE matmuls; a single ones-matmul per (j,head) reduces partitions and
broadcasts. Diagonal S/O matmuls and exp are narrowed to the valid
query range. Exp batches pairs of s-chunks into [128,1024] ACTIVATEs.
Dummy warm-up matmuls at t=0 hold the PE HAM clock-gate at K=8/8.
"""
from contextlib import ExitStack

import numpy as np

import concourse.bass as bass
import concourse.mybir as mybir
import concourse.tile as tile
from concourse import bacc
from concourse.bass_utils import run_bass_kernel_spmd

B, T, D = 2, 2048, 2048
H, KV, HD = 16, 4, 128
R = H // KV                  # 4 query heads per kv head (per core)
GC = R * HD                  # 512 query-proj cols per core
THETA = 10000.0
TQ = 512                     # q-tile size
NJ = T // TQ                 # 4 q-tiles
ND = D // 128                # 16 contraction chunks
SCALE = float(HD) ** -0.5

F32 = mybir.dt.float32
BF16 = mybir.dt.bfloat16
AF = mybir.ActivationFunctionType

_CACHED_NC = None


def _build_nc():
    nc = bacc.Bacc("TRN2", target_bir_lowering=False, debug=False, num_devices=8)

    # xT: chunk d at cols [d*T, (d+1)*T), natural token order within chunk
    xT = nc.dram_tensor("xT", [128, ND * T], BF16, kind="ExternalInput").ap()
    wq = nc.dram_tensor("wq", [128, ND * GC], BF16, kind="ExternalInput").ap()
    wk = nc.dram_tensor("wk", [128, ND * HD], BF16, kind="ExternalInput").ap()
    wv = nc.dram_tensor("wv", [128, ND * HD], BF16, kind="ExternalInput").ap()
    wo = nc.dram_tensor("wo", [128, R * D], BF16, kind="ExternalInput").ap()
    cosT = nc.dram_tensor("cosT", [HD, T], BF16, kind="ExternalInput").ap()
    sinT = nc.dram_tensor("sinT", [HD, T], BF16, kind="ExternalInput").ap()
    tri = nc.dram_tensor("tri", [128, 128], BF16, kind="ExternalInput").ap()
    out = nc.dram_tensor("out", [T, D], F32, kind="ExternalOutput").ap()

    with tile.TileContext(nc) as tc, ExitStack() as ctx:
        res = ctx.enter_context(tc.tile_pool(name="res", bufs=1))
        sb = ctx.enter_context(tc.tile_pool(name="sb", bufs=2))
        pp = ctx.enter_context(tc.tile_pool(name="pp", bufs=2, space="PSUM"))

        # ---- warm-up source (no DMA dependency) + PE HAM warm-up ----
        warm = res.tile([128, 512], BF16)
        nc.vector.memset(warm[:], 0.0)
        ones_c = res.tile([128, 128], BF16)
        nc.vector.memset(ones_c[:], 1.0)
        ident = res.tile([128, 128], BF16)
        from concourse.masks import make_identity
        identf = res.tile([128, 128], F32)
        make_identity(nc, identf[:])
        nc.vector.tensor_copy(ident[:], identf[:])
        for w in range(14):
            wm_ps = pp.tile([128, 512], F32, tag="ps", bufs=2, name=f"warm{w}")
            nc.tensor.matmul(wm_ps[:], ones_c[:], warm[:], start=True, stop=True)

        # ---- resident weights / tables ----
        # input stream (xT, cos, sin) on the sync queue; weights on the
        # scalar / vector engines' queues so they flow in parallel.
        wk_sb = res.tile([128, ND * HD], BF16)
        nc.scalar.dma_start(wk_sb[:], wk[:])
        wv_sb = res.tile([128, ND * HD], BF16)
        nc.scalar.dma_start(wv_sb[:], wv[:])
        wq_sb = res.tile([128, ND * GC], BF16)   # chunk d at cols [d*GC,(d+1)*GC)
        for qd in range(4):
            nc.scalar.dma_start(wq_sb[:, qd * 4 * GC:(qd + 1) * 4 * GC],
                                wq[:, qd * 4 * GC:(qd + 1) * 4 * GC])
        tri_sb = res.tile([128, 128], BF16)
        nc.gpsimd.dma_start(tri_sb[:], tri[:])
        wo_sb = res.tile([128, R * D], BF16)     # head h rows at cols [h*D,(h+1)*D)
        nc.gpsimd.dma_start(wo_sb[:], wo[:])

        kT_sb = res.tile([128, T], BF16)         # K^T resident, filled per j
        v_sb = res.tile([128, T], BF16)          # V natural, chunk c at cols c*128

        def rope(dst, ps, cosj, sinj):
            # dst = ps * cos + rotate_half(ps) * sin  (partition dim = head dim)
            rot = sb.tile([128, TQ], BF16, tag="rot", bufs=2)
            nc.scalar.mul(rot[0:64, :], ps[64:128, :], -1.0)
            nc.scalar.copy(rot[64:128, :], ps[0:64, :])
            tmp = sb.tile([128, TQ], F32, tag="ropetmp", bufs=2)
            nc.vector.tensor_mul(tmp[:], rot[:], sinj[:])
            m1 = sb.tile([128, TQ], F32, tag="ropem1", bufs=2)
            nc.vector.tensor_mul(m1[:], ps[:], cosj[:])
            nc.vector.tensor_add(dst, m1[:], tmp[:])

        for j in range(NJ):
            q0 = j * TQ
            # ---- stage inputs for this q/s tile ----
            xts = []
            for d in range(ND):
                xt = sb.tile([128, TQ], BF16, tag="xt", bufs=32)
                nc.sync.dma_start(xt[:], xT[:, d * T + q0:d * T + q0 + TQ])
                xts.append(xt)
            cosj = sb.tile([128, TQ], BF16, tag="cos", bufs=2)
            nc.sync.dma_start(cosj[:], cosT[:, q0:q0 + TQ])
            sinj = sb.tile([128, TQ], BF16, tag="sin", bufs=2)
            nc.sync.dma_start(sinj[:], sinT[:, q0:q0 + TQ])

            # ---- A1: K^T and V^T for s-tile j ----
            k_ps = pp.tile([128, TQ], F32, tag="pa", bufs=2)
            vt_ps = pp.tile([128, TQ], F32, tag="pa", bufs=2)
            for d in range(ND):
                nc.tensor.matmul(k_ps[:], wk_sb[:, d * HD:(d + 1) * HD], xts[d][:],
                                 start=(d == 0), stop=(d == ND - 1))
            for d in range(ND):
                nc.tensor.matmul(vt_ps[:], wv_sb[:, d * HD:(d + 1) * HD], xts[d][:],
                                 start=(d == 0), stop=(d == ND - 1))
            rope(kT_sb[:, q0:q0 + TQ], k_ps, cosj, sinj)
            vt_sbt = sb.tile([128, TQ], BF16, tag="vtsb", bufs=2)
            nc.vector.tensor_copy(vt_sbt[:], vt_ps[:])
            for c4 in range(4):
                ptt = pp.tile([128, 128], BF16, tag="pa", bufs=2)
                nc.tensor.transpose(ptt[:], vt_sbt[:, c4 * 128:(c4 + 1) * 128], ident[:])
                nc.vector.tensor_copy(v_sb[:, (4 * j + c4) * 128:(4 * j + c4 + 1) * 128], ptt[:])

            # ---- A2: Q^T per head + rope ----
            q_tiles = []
            for h in range(R):
                q_ps = pp.tile([128, TQ], F32, tag="pa", bufs=2)
                for d in range(ND):
                    nc.tensor.matmul(
                        q_ps[:], wq_sb[:, d * GC + h * 128:d * GC + (h + 1) * 128],
                        xts[d][:], start=(d == 0), stop=(d == ND - 1))
                qh = sb.tile([128, TQ], BF16, tag="qsb", bufs=5)
                rope(qh[:], q_ps, cosj, sinj)
                q_tiles.append(qh)

            # ---- B: causal attention per head ----
            o_tiles = []
            ncf = 4 * j              # full (below-diagonal) s-chunks
            for h in range(R):
                o_ps = pp.tile([128, TQ], F32, tag="po", bufs=2)
                acc2 = None
                # full chunks, paired into [128,1024] groups
                for bi in range(ncf // 2):
                    c0 = 2 * bi
                    s_grp = pp.tile([128, 2 * TQ], F32, tag="ps", bufs=2)
                    for cc in range(2):
                        nc.tensor.matmul(s_grp[:, cc * TQ:(cc + 1) * TQ],
                                         kT_sb[:, (c0 + cc) * 128:(c0 + cc + 1) * 128],
                                         q_tiles[h][:], start=True, stop=True)
                    p_grp = sb.tile([128, 2 * TQ], BF16, tag="psb", bufs=3)
                    nc.scalar.activation(p_grp[:], s_grp[:], AF.Exp, scale=SCALE)
                    for cc in range(2):
                        nc.tensor.matmul(o_ps[:], v_sb[:, (c0 + cc) * 128:(c0 + cc + 1) * 128],
                                         p_grp[:, cc * TQ:(cc + 1) * TQ],
                                         start=(c0 + cc == 0), stop=False)
                    if acc2 is None:
                        acc2 = sb.tile([128, 2 * TQ], BF16, tag="acc2", bufs=2)
                        nc.vector.tensor_copy(acc2[:], p_grp[:])
                    else:
                        nc.vector.tensor_add(acc2[:], acc2[:], p_grp[:])
                # diagonal chunks m=0..3: valid q cols [m*128, 512)
                acc1 = sb.tile([128, TQ], BF16, tag="acc1", bufs=2)
                for m in range(4):
                    c = 4 * j + m
                    w = TQ - m * 128
                    s_d = pp.tile([128, 2 * TQ], F32, tag="ps", bufs=2)
                    nc.tensor.matmul(s_d[:, 0:w],
                                     kT_sb[:, c * 128:(c + 1) * 128],
                                     q_tiles[h][:, m * 128:TQ], start=True, stop=True)
                    p_d = sb.tile([128, TQ], BF16, tag="psb", bufs=3)
                    nc.scalar.activation(p_d[:, 0:w], s_d[:, 0:w], AF.Exp, scale=SCALE)
                    nc.vector.tensor_mul(p_d[:, 0:128], p_d[:, 0:128],
                                         tri_sb[:])
                    nc.tensor.matmul(o_ps[:, m * 128:TQ],
                                     v_sb[:, c * 128:(c + 1) * 128],
                                     p_d[:, 0:w],
                                     start=(c == 0), stop=(m == 3))
                    if m == 0:
                        nc.vector.tensor_copy(acc1[:], p_d[:])
                    else:
                        nc.vector.tensor_add(acc1[:, m * 128:TQ],
                                             acc1[:, m * 128:TQ],
                                             p_d[:, 0:w])
                # sigma = partition-reduce+broadcast of (acc2.lo+acc2.hi+acc1)
                if acc2 is not None:
                    nc.vector.tensor_add(acc1[:], acc1[:], acc2[:, 0:TQ])
                    nc.vector.tensor_add(acc1[:], acc1[:], acc2[:, TQ:2 * TQ])
                sg_ps = pp.tile([128, TQ], F32, tag="pa", bufs=2)
                nc.tensor.matmul(sg_ps[:], ones_c[:], acc1[:], start=True, stop=True)
                rcb = sb.tile([128, TQ], F32, tag="rcb", bufs=2)
                nc.vector.reciprocal_approx_fast(rcb[:], sg_ps[:])
                oh = sb.tile([128, TQ], BF16, tag="osb", bufs=6)
                nc.vector.tensor_mul(oh[:], o_ps[:], rcb[:])
                o_tiles.append(oh)

            # ---- C: output projection for q-tile j ----
            for qs in range(4):
                for n in range(NJ):
                    pc = pp.tile([128, 512], F32, tag="po", bufs=2)
                    for h in range(R):
                        nc.tensor.matmul(
                            pc[:], o_tiles[h][:, qs * 128:(qs + 1) * 128],
                            wo_sb[:, h * D + n * 512:h * D + (n + 1) * 512],
                            start=(h == 0), stop=(h == R - 1))
                    ob = sb.tile([128, 512], F32, tag="ob", bufs=3)
                    nc.scalar.copy(ob[:], pc[:])
                    nc.gpsimd.dma_start(
                        out[q0 + qs * 128:q0 + (qs + 1) * 128, n * 512:(n + 1) * 512],
                        ob[:])

    nc.compile()
    return nc


def _get_nc():
    global _CACHED_NC
    if _CACHED_NC is None:
        _CACHED_NC = _build_nc()
    return _CACHED_NC


def _rope_tables_T():
    inv_freq = (1.0 / (THETA ** (np.arange(0, HD, 2, dtype=np.float32) / HD))).astype(np.float32)
    pos = np.arange(T, dtype=np.float32)
    freqs = np.outer(pos, inv_freq).astype(np.float32)      # [T, HD/2]
    emb = np.concatenate([freqs, freqs], axis=-1)           # [T, HD]
    return (np.cos(emb).T.copy(), np.sin(emb).T.copy())     # [HD, T] f32


def kernel(x, Wq, Wk, Wv, Wo, _trace=False):
    import ml_dtypes
    BFNP = ml_dtypes.bfloat16
    x = np.asarray(x, dtype=np.float32)

    cosT, sinT = _rope_tables_T()
    # tri[i, jj] = 1 if jj >= i (keep) else 0, for the diagonal 128-block
    i_ = np.arange(128)[:, None]
    jj_ = np.arange(128)[None, :]
    tri = (jj_ >= i_).astype(BFNP)

    def chunkT(w):  # [ND*128, C] -> [128, ND*C] with chunk d at cols [d*C,(d+1)*C)
        nd = w.shape[0] // 128
        return np.ascontiguousarray(
            w.reshape(nd, 128, -1).transpose(1, 0, 2).reshape(128, -1)).astype(BFNP)

    in_maps = []
    for core in range(8):
        b, g = core // KV, core % KV
        # xT chunked: [128, ND*T], chunk d at cols [d*T,(d+1)*T)
        xb = x[b].T.reshape(ND, 128, T).transpose(1, 0, 2).reshape(128, ND * T)
        in_maps.append({
            "xT": np.ascontiguousarray(xb).astype(BFNP),
            "wq": chunkT(np.asarray(Wq)[:, g * GC:(g + 1) * GC]),
            "wk": chunkT(np.asarray(Wk)[:, g * HD:(g + 1) * HD]),
            "wv": chunkT(np.asarray(Wv)[:, g * HD:(g + 1) * HD]),
            "wo": chunkT(np.asarray(Wo)[g * GC:(g + 1) * GC, :]),
            "cosT": cosT.astype(BFNP), "sinT": sinT.astype(BFNP), "tri": tri,
        })

    nc = _get_nc()
    res = run_bass_kernel_spmd(nc, in_maps, core_ids=list(range(8)), trace=_trace)

    outp = np.zeros((B, T, D), dtype=np.float32)
    for core in range(8):
        b = core // KV
        outp[b] += res.results[core]["out"]
    if _trace:
        kernel._last_exec_time_ns = res.exec_time_ns
        kernel._last_trace = res.instructions_and_trace
    return outp
